# revision 29
# baseline (speedup 1.0000x reference)
"""Margin-based triplet criterion (loss_fn) on 8 TRN2 NeuronCores.

Strategy (Gram-matrix formulation, fine-grained tournament coverage):
  Every pairwise squared distance is d^2(u,v) = s[u] + s[v] - 2*x_u.x_v, so
  the device-heavy quantity is the dot product x_u.x_v.  Instead of gathering
  3 full embedding rows per triplet (24 MB/core of DMA), each core computes a
  slice of the Gram matrix G = X X^T on the Tensor engine; the per-pair
  scalar lookups and the O(T) elementwise hinge epilogue are cheap index
  work done on the host (which already owns the O(T) triplet routing).

  Rows are split into 32 blocks of 128; core j owns column blocks
  {4j..4j+3}.  Column block a covers row blocks {a..a+16} (mod 32) -- a
  tournament schedule on 32 nodes: every unordered block pair is covered
  (distance-16 pairs twice; the host balances those), so the fleet computes
  only 17/32 of B^2 Gram entries -- finer than an 8-block tournament and 15%
  less than 5/8 -- while each core still loads the same 2560 rows of X.
  Per core that is 68 tiles of [128 x 128] = 1.11 MB of G, 1.9 GF of matmul.

  Per core: X^T arrives as fp8 (e4m3) in 5 chunks; tiles are processed in
  ascending local-row order so the input stream is consumed progressively.
  Each PSUM allocation ([128, 8, 128] = 2 banks) holds 8 tiles as two
  4-tile accumulation groups (one per 2KB bank, lazy zero on first touch),
  filled by DoubleRow matmuls (K=256, 2 per tile).  A few dummy matmuls at
  t=0 ramp the PE p-state while the DMA front lands.  PSUM is downconverted
  to fp8 with scale 1/8 (G stores dot/8; e4m3 max is 240 and self-dots reach
  ~1100) alternating Scalar/Vector engines, and the G slice is the kernel
  output (5 DRAM tensors so writes don't false-serialize), streamed out via
  the idle Pool SWDGE queue while later tiles compute; the last two leave
  via SP/HWDGE to shorten the tail.

  Host: routes each (anchor, partner) pair to its covering core, looks the
  dot up in that core's returned G slice, forms d = sqrt(max(ssum-2dot,0)
  + eps), hinge losses, the active-pair OR, and the final division --
  all O(T) numpy, same order as the routing prep itself.
"""

import numpy as np
import ml_dtypes
from contextlib import ExitStack

import concourse.bass as bass
import concourse.bacc as bacc
import concourse.tile as tile
from concourse import mybir
from concourse.bass_utils import run_bass_kernel_spmd

N_CORES = 8
B, D, T, C = 4096, 512, 65536, 100
MARGIN = 0.2
EPS = 1e-8

NB = 32                   # row/column blocks of 128
CPB = 4                   # column blocks per core
NS = 17                   # row blocks covered per column block (incl. self)
ROWS_L = 2560             # rows of X loaded per core (20 blocks of 128)
KB = 4                    # contraction chunks of 128 (D = 512)
GSCALE = 8.0              # G stores dot/GSCALE (fp8 e4m3 max ~240)
N_TILES = CPB * NS        # 68 G tiles of [128 x 128] per core

# Tile processing order: ascending local row block (c + s) so the lhs DMA
# chunks are consumed progressively; this order also defines the G layout.
TILE_ORDER = sorted(((c, s) for c in range(CPB) for s in range(NS)),
                    key=lambda cs: (cs[0] + cs[1], cs[0]))
TILE_POS = {cs: i for i, cs in enumerate(TILE_ORDER)}
# PSUM allocation sizes (first small so the copy lanes prime early) and
# the grouping of tiles into output tensors (last ones small for the tail).
ALLOC_SIZES = [4, 8, 8, 8, 8, 8, 8, 8, 8]
GW_TILES = [12, 16, 16, 8, 8, 8]

f32 = mybir.dt.float32
fp8 = mybir.dt.float8e4
np_fp8 = ml_dtypes.float8_e4m3

_CACHE = {}


def _build_nc():
    nc = bacc.Bacc(
        "TRN2", target_bir_lowering=False, debug=False,
        enable_asserts=False, num_devices=N_CORES,
    )
    lhs = nc.dram_tensor("lhs", [128, KB, ROWS_L], fp8, kind="ExternalInput")
    gouts = [
        nc.dram_tensor(f"gout{w}", [128, GW_TILES[w], 128], fp8,
                       kind="ExternalOutput")
        for w in range(len(GW_TILES))
    ]

    with tile.TileContext(nc) as tc, ExitStack() as ctx:
        const_pool = ctx.enter_context(tc.tile_pool(name="const", bufs=1))
        gsb_pool = ctx.enter_context(tc.tile_pool(name="gsb", bufs=5))
        psum_pool = ctx.enter_context(
            tc.tile_pool(name="psum", bufs=4, space="PSUM"))

        # PE p-state warmup: dummy matmuls while the lhs DMA front lands so
        # the real matmuls run at the fully-ramped clock.
        warm_sb = const_pool.tile([128, 512], mybir.dt.bfloat16)
        nc.vector.memset(warm_sb[:], 0.0)
        ps_warm = psum_pool.tile([128, 8, 128], f32, tag="ps", name="ps_warm")
        for _ in range(4):
            nc.tensor.matmul(ps_warm[:, 0:4, :], lhsT=warm_sb[:, 0:128],
                             rhs=warm_sb[:], start=True, stop=True)

        # Local rows 0..2559 = global blocks {4j..4j+19}; the first CPB
        # local blocks double as the rhs (this core's column blocks).
        lhs_sb = const_pool.tile([128, KB, ROWS_L], fp8)
        for ch in range(5):
            sl = slice(ch * 512, (ch + 1) * 512)
            nc.sync.dma_start(lhs_sb[:, :, sl], lhs[:, :, sl])

        base = 0
        gsb_fill = 0
        g_t = None
        wi = 0
        for p, n in enumerate(ALLOC_SIZES):
            ids = list(range(base, base + n))
            base += n
            ps = psum_pool.tile([128, 8, 128], f32, tag="ps", name=f"ps_{p}")
            # Accumulation groups of up to 4 tiles, one per 2KB PSUM bank;
            # the bank is lazily zeroed on each tile's first matmul.
            for bi in range(0, n, 4):
                grp = ids[bi:bi + 4]
                for k, tid in enumerate(grp):
                    c, s = TILE_ORDER[tid]
                    rloc = c + s
                    for kc in range(2):
                        nc.tensor.matmul(
                            ps[:, bi + k, :],
                            lhsT=lhs_sb[:, 2 * kc:2 * kc + 2,
                                        rloc * 128:(rloc + 1) * 128],
                            rhs=lhs_sb[:, 2 * kc:2 * kc + 2,
                                       c * 128:(c + 1) * 128],
                            start=(k == 0 and kc == 0),
                            stop=(k == len(grp) - 1 and kc == 1),
                            perf_mode=mybir.MatmulPerfMode.DoubleRow,
                        )
            # G stores dot/GSCALE to fit fp8 e4m3 (self-dots ~1100).
            if gsb_fill == 0:
                g_t = gsb_pool.tile([128, GW_TILES[wi], 128], fp8,
                                    tag=f"gt{GW_TILES[wi]}", name=f"g_{wi}")
            dst = g_t[:, gsb_fill:gsb_fill + n, :]
            if p % 2 == 0:
                nc.scalar.activation(
                    out=dst, in_=ps[:, 0:n, :],
                    func=mybir.ActivationFunctionType.Copy,
                    scale=1.0 / GSCALE)
            else:
                nc.vector.tensor_scalar_mul(dst, ps[:, 0:n, :], 1.0 / GSCALE)
            gsb_fill += n
            if gsb_fill == GW_TILES[wi]:
                # Early groups via the idle Pool SWDGE queue; the last two
                # via SP/HWDGE (free again, lower issue latency).
                eng = nc.sync if wi >= len(GW_TILES) - 2 else nc.gpsimd
                eng.dma_start(gouts[wi][:], g_t[:])
                wi += 1
                gsb_fill = 0

    nc.compile()
    return nc


def _pack_kt(rows_x):
    """[R, 512] (row-major) -> [128, 4, R] SBUF weight layout (p, kb, r)."""
    r = rows_x.shape[0]
    return np.ascontiguousarray(
        rows_x.T.reshape(KB, 128, r).transpose(1, 0, 2))


def _prep_inputs(batch, beta, labels, triplets):
    batch = np.asarray(batch, dtype=np.float32)
    beta = np.asarray(beta, dtype=np.float32)
    labels = np.asarray(labels).astype(np.int64)
    triplets = np.asarray(triplets).astype(np.int64)

    xb = batch.astype(np_fp8)
    s = (xb.astype(np.float64) ** 2).sum(axis=1)          # [B] exact on fp8
    assert s.max() / GSCALE < 235.0, "G fp8 range"
    ia = triplets[:, 0]
    bt = beta[labels[ia]].astype(np.float64)              # [T]

    # Route each (u, v) pair to the core that computed its Gram entry.
    # Column block a = u>>7 covers row blocks {a..a+16} (mod 32).
    tile_pos = np.full((CPB, NS), -1, dtype=np.int64)
    for (c, s_), i in TILE_POS.items():
        tile_pos[c, s_] = i
    routed = {}
    for kind, v_arr in (("ap", triplets[:, 1]), ("an", triplets[:, 2])):
        u = ia
        v = v_arr
        a = (u >> 7).astype(np.int64)
        b = (v >> 7).astype(np.int64)
        d = (b - a) & 31
        use_a = d <= 15
        # d == 16 pairs are covered by both sides; balance them per core.
        core0 = np.where(use_a, a >> 2, b >> 2)
        cnt = np.bincount(core0[d != 16], minlength=8)
        sel16 = np.zeros(len(u), dtype=bool)
        for g0 in range(16):
            g1 = g0 + 16
            grp = np.where((d == 16) & ((a == g0) | (a == g1)))[0]
            n = len(grp)
            ca, cb = g0 >> 2, g1 >> 2
            x = int(np.clip((cnt[cb] + n - cnt[ca]) // 2, 0, n))
            # first x pairs: use their own a-side if a==g0 else flip
            sel16[grp[:x]] = True   # route to core ca (col block g0)
            cnt[ca] += x
            cnt[cb] += n - x
        # for d==16: route to ca means col endpoint in block g0
        is16 = d == 16
        use_a = np.where(is16, np.where(sel16, a < 16, ~(a < 16)), use_a)
        # careful: "col block g0 (<16)" — col endpoint is u if a==g0 else v
        col = np.where(use_a, u, v)
        row = np.where(use_a, v, u)
        ca_ = (col >> 7).astype(np.int64)
        s_idx = ((row >> 7) - ca_) & 31
        assert s_idx.max() <= NS - 1, "coverage violation"
        core = ca_ >> 2
        c_loc = ca_ & 3
        gtile = tile_pos[c_loc, s_idx]
        off = (row & 127) * (N_TILES * 128) + gtile * 128 + (col & 127)
        ssum_v = s[row] + s[col]
        routed[kind] = (core, off, ssum_v)

    in_maps = []
    for j in range(N_CORES):
        blocks = [(4 * j + t) % NB for t in range(20)]
        rows_all = np.concatenate(
            [np.arange(128 * b_, 128 * b_ + 128) for b_ in blocks])
        in_maps.append({"lhs": _pack_kt(xb[rows_all])})       # [128,4,2560]
    return in_maps, (routed, bt)


def _finalize(results, meta):
    routed, bt = meta
    g_flat = np.stack(
        [np.concatenate([np.asarray(r[f"gout{w}"]).astype(np.float32)
                         .reshape(128, -1) for w in range(len(GW_TILES))],
                        axis=1).reshape(-1)
         for r in results])

    active = {}
    total = np.float64(0.0)
    for kind in ("ap", "an"):
        core, off, ssum_v = routed[kind]
        dot2 = g_flat[core, off].astype(np.float64) * (2.0 * GSCALE)
        dd = np.sqrt(np.maximum(ssum_v - dot2, 0.0) + EPS)
        thr = bt - MARGIN if kind == "ap" else bt + MARGIN
        h = np.maximum(dd - thr if kind == "ap" else thr - dd, 0.0)
        total += h.sum()
        active[kind] = h > 0.0
    cnt = np.float32((active["ap"] | active["an"]).sum())
    total = np.float32(total)
    if cnt > 0.0:
        loss = total / max(cnt, np.float32(1.0))
    else:
        loss = total
    return np.float32(loss)


def run_hw(batch, beta, labels, triplets, trace=False, **kw):
    if "nc" not in _CACHE:
        _CACHE["nc"] = _build_nc()
    nc = _CACHE["nc"]
    in_maps, meta = _prep_inputs(batch, beta, labels, triplets)
    res = run_bass_kernel_spmd(nc, in_maps, list(range(N_CORES)), trace=trace,
                               **kw)
    return _finalize(res.results, meta), res


def kernel(batch, beta, labels, triplets):
    loss, _ = run_hw(batch, beta, labels, triplets)
    return loss
